# revision 10
# baseline (speedup 1.0000x reference)
"""DeltaNet (chunked delta-rule linear attention) on 8 TRN2 NeuronCores.

Sharding: batch x head-group. core c -> batch c//4, heads (c%4)*4 .. +4
(2 head-pairs per core). Per-head recurrent state stays core-local.

Numerics: the reference recurrence explodes in fp32 (|out| up to 3e38,
97.8% nonfinite); matching it requires fp32-faithful compute. All
recurrence matmuls are fp32 (PE, 4cyc/row); projections use float32r
(1cyc/row at N=512, ~1.5e-4 rounding - within the chaos envelope).

UT transform per chunk (C=64): with D = -strict_tril(beta_i * k k^T),
  T = (I-D)^{-1} = (I+D)(I+D^2)(I+D^4)(I+D^8)(I+D^16)(I+D^32)
applied transposed: TT = prod (I+DT^p); chain RT <- RT + DTp @ RT
(lhsT = Dp), with dual power chains (DTp: lhsT=D_{p/2} rhs=DT_{p/2};
Dp: lhsT=DT_{p/2} rhs=D_{p/2}).
Two heads pack per 128-partition tile as diagonal PSUM quadrants
(explicit tile_position hard-faults on fp32; implicit diagonal
placement is safe and ~2x faster than serial 64^3 matmuls).
"""

import os
import numpy as np

import concourse.bass as bass
import concourse.mybir as mybir
import concourse.tile as tile
from concourse import bacc
from concourse import bass_utils

F32 = mybir.dt.float32
F32R = mybir.dt.float32 if os.environ.get("PROJ_FP32") else mybir.dt.float32r
ALU = mybir.AluOpType
ACTF = mybir.ActivationFunctionType

B, L, DIM, H = 2, 4096, 1024, 16
DK = DV = 64
C = 64
NCHUNK = L // C
N_CORES = 8
HPC = 4                     # heads per core
NPAIR = 2
KT = DIM // 128             # 8
LT512 = L // 512
LT128 = L // 128

LAST_EXEC_NS = [None]


def _masks():
    sl = np.tril(np.ones((C, C), np.float32), -1)
    tu = np.triu(np.ones((C, C), np.float32))    # incl diag
    z = np.zeros((C, C), np.float32)
    m_d = np.block([[sl, z], [z, sl]])
    m_at = np.block([[tu, z], [z, tu]])
    i128 = np.eye(128, dtype=np.float32)
    ist = np.concatenate([np.eye(C, dtype=np.float32)] * 2, 0)
    i4 = np.eye(4, dtype=np.float32)
    return m_d, m_at, i128, ist, i4


def _build():
    nc = bacc.Bacc("TRN2", target_bir_lowering=False, debug=False,
                   num_devices=N_CORES)
    d_hsT = nc.dram_tensor("hsT", [DIM, L], F32, kind="ExternalInput").ap()
    d_wq = nc.dram_tensor("wq", [DIM, 256], F32, kind="ExternalInput").ap()
    d_wkv = nc.dram_tensor("wkv", [DIM, 512], F32, kind="ExternalInput").ap()
    d_wbn = nc.dram_tensor("wbn", [DIM, 4], F32, kind="ExternalInput").ap()
    d_md = nc.dram_tensor("m_d", [128, 128], F32, kind="ExternalInput").ap()
    d_mat = nc.dram_tensor("m_at", [128, 128], F32, kind="ExternalInput").ap()
    d_i128 = nc.dram_tensor("i128", [128, 128], F32, kind="ExternalInput").ap()
    d_ist = nc.dram_tensor("ist", [128, 64], F32, kind="ExternalInput").ap()
    d_i4 = nc.dram_tensor("i4", [4, 4], F32, kind="ExternalInput").ap()
    d_o = nc.dram_tensor("o", [HPC, L, DV], F32, kind="ExternalOutput").ap()

    MM = nc.tensor.matmul
    with tile.TileContext(nc) as tc:
        with tc.tile_pool(name="per", bufs=1) as per:
            # constants
            m_d = per.tile([128, 128], F32, tag="m_d")
            m_at = per.tile([128, 128], F32, tag="m_at")
            i128 = per.tile([128, 128], F32, tag="i128")
            ist = per.tile([128, 64], F32, tag="ist")
            i4 = per.tile([4, 4], F32, tag="i4")
            for t, d in ((m_d, d_md), (m_at, d_mat), (i128, d_i128),
                         (ist, d_ist), (i4, d_i4)):
                nc.sync.dma_start(t[:], d)

            # persistent activations
            qT = [per.tile([128, L], F32, tag=f"qT{p}", name=f"qT{p}") for p in range(NPAIR)]
            kT = [per.tile([128, L], F32, tag=f"kT{p}", name=f"kT{p}") for p in range(NPAIR)]
            kv_tok = per.tile([128, LT128 * 512], F32, tag="kv_tok")
            bTq = per.tile([4, L], F32, tag="bTq")   # beta_neg feat-major
            S = [per.tile([128, DV], F32, tag=f"S{p}", name=f"S{p}") for p in range(NPAIR)]

            # ============ P0: projections (f32r) ============
            with (
                tc.tile_pool(name="wp", bufs=1) as wp,
                tc.tile_pool(name="stg", bufs=2) as stg,
                tc.tile_pool(name="ps0", bufs=1, space="PSUM") as ps0,
            ):
                wq_r = wp.tile([128, KT * 256], F32R, tag="wq_r")
                wkv_r = wp.tile([128, KT * 512], F32R, tag="wkv_r")
                wbn_r = wp.tile([128, KT * 4], F32R, tag="wbn_r")
                for kt in range(KT):
                    ks = slice(kt * 128, (kt + 1) * 128)
                    wq_f = stg.tile([128, 256], F32, tag="wq_f")
                    nc.sync.dma_start(wq_f[:], d_wq[ks, :])
                    nc.vector.tensor_copy(wq_r[:, kt * 256:(kt + 1) * 256], wq_f[:])
                    wkv_f = stg.tile([128, 512], F32, tag="wkv_f")
                    nc.sync.dma_start(wkv_f[:], d_wkv[ks, :])
                    nc.vector.tensor_copy(wkv_r[:, kt * 512:(kt + 1) * 512], wkv_f[:])
                    wbn_f = stg.tile([128, 4], F32, tag="wbn_f")
                    nc.sync.dma_start(wbn_f[:], d_wbn[ks, :])
                    nc.vector.tensor_copy(wbn_r[:, kt * 4:(kt + 1) * 4], wbn_f[:])

                # feat pass
                for lt in range(LT512):
                    ls = slice(lt * 512, (lt + 1) * 512)
                    pq = [ps0.tile([128, 512], F32, tag=f"p_q{p}", name=f"p_q{p}")
                          for p in range(NPAIR)]
                    pk = [ps0.tile([128, 512], F32, tag=f"p_k{p}", name=f"p_k{p}")
                          for p in range(NPAIR)]
                    pb = ps0.tile([4, 512], F32, tag="p_b")
                    for kt in range(KT):
                        hf = stg.tile([128, 512], F32, tag="hs_f", bufs=3)
                        nc.sync.dma_start(hf[:], d_hsT[kt * 128:(kt + 1) * 128, ls])
                        hr = stg.tile([128, 512], F32R, tag="hs_r", bufs=3)
                        nc.vector.tensor_copy(hr[:], hf[:])
                        f, l8 = (kt == 0), (kt == KT - 1)
                        for p in range(NPAIR):
                            o_ = kt * 256 + p * 128
                            MM(pq[p][:], wq_r[:, o_:o_ + 128], hr[:],
                               start=f, stop=l8)
                            o_ = kt * 512 + p * 128
                            MM(pk[p][:], wkv_r[:, o_:o_ + 128], hr[:],
                               start=f, stop=l8)
                        MM(pb[:], wbn_r[:, kt * 4:(kt + 1) * 4], hr[:],
                           start=f, stop=l8)
                    for p in range(NPAIR):
                        nc.scalar.activation(qT[p][:, ls], pq[p][:], ACTF.Silu)
                        nc.vector.tensor_scalar_mul(qT[p][:, ls], qT[p][:, ls], 0.125)
                        nc.scalar.activation(kT[p][:, ls], pk[p][:], ACTF.Silu)
                    nc.vector.tensor_copy(bTq[:, ls], pb[:])

                # token pass
                for lt in range(LT128):
                    ls = slice(lt * 128, (lt + 1) * 128)
                    pkv = ps0.tile([128, 512], F32, tag="p_kv", bufs=2)
                    for kt in range(KT):
                        hf = stg.tile([128, 128], F32, tag="hs_tf", bufs=3)
                        nc.sync.dma_start(hf[:], d_hsT[kt * 128:(kt + 1) * 128, ls])
                        hr = stg.tile([128, 128], F32R, tag="hs_tr", bufs=3)
                        nc.gpsimd.tensor_copy(hr[:], hf[:])
                        MM(pkv[:], hr[:], wkv_r[:, kt * 512:(kt + 1) * 512],
                           start=(kt == 0), stop=(kt == KT - 1))
                    cs0 = lt * 512
                    nc.scalar.activation(kv_tok[:, cs0:cs0 + 256], pkv[:, 0:256],
                                         ACTF.Silu)
                    nc.vector.tensor_copy(kv_tok[:, cs0 + 256:cs0 + 512],
                                          pkv[:, 256:512])

            # ============ P1: transform + scan ============
            with (
                tc.tile_pool(name="wrk", bufs=2) as wrk,
                tc.tile_pool(name="ps", bufs=8, space="PSUM") as ps,
            ):
                for p in range(NPAIR):
                    nc.vector.memset(S[p][:], 0.0)

                POW = [2, 4, 8, 16, 32]
                Q0, Q1 = slice(0, 64), slice(64, 128)
                for c in range(NCHUNK):
                    cs = slice(c * C, (c + 1) * C)
                    half = (c % 2) * 64
                    blk = (c // 2) * 512
                    bq = bTq[:, cs]
                    for p in range(NPAIR):
                        h1, h2 = 2 * p, 2 * p + 1

                        # KB tile [128, 192] = kbneg | vb | k
                        kb = wrk.tile([128, 192], F32, tag="kb", bufs=3)
                        for h, QD in ((h1, Q0), (h2, Q1)):
                            nc.sync.dma_start(
                                kb[QD, 128:192],
                                kv_tok[half:half + 64,
                                       blk + h * 64:blk + h * 64 + 64])
                            nc.sync.dma_start(
                                kb[QD, 64:128],
                                kv_tok[half:half + 64,
                                       blk + 256 + h * 64:blk + 256 + h * 64 + 64])
                        pbt = ps.tile([128, 512], F32, tag="pp")
                        nc.tensor.transpose(pbt[Q0, 0:4], bq, i4[:])
                        bchunk = wrk.tile([64, 4], F32, tag="bchunk", bufs=3)
                        nc.vector.tensor_copy(bchunk[:], pbt[Q0, 0:4])
                        bmix = wrk.tile([128, 1], F32, tag="bmix", bufs=3)
                        nc.vector.tensor_copy(bmix[Q0, :], bchunk[:, h1:h1 + 1])
                        nc.sync.dma_start(bmix[Q1, :], bchunk[:, h2:h2 + 1])
                        nc.gpsimd.tensor_scalar(kb[:, 0:64], kb[:, 128:192],
                                                bmix[:, 0:1], None, op0=ALU.mult)
                        nc.gpsimd.tensor_scalar(kb[:, 64:128], kb[:, 64:128],
                                                bmix[:, 0:1], -1.0,
                                                op0=ALU.mult, op1=ALU.mult)

                        # kk cross -> D
                        pkk = ps.tile([128, 512], F32, tag="pp")
                        MM(pkk[Q0, 0:64], kT[p][Q0, cs], kT[p][Q0, cs],
                           start=True, stop=True, skip_group_check=True)
                        MM(pkk[Q1, 64:128], kT[p][Q1, cs], kT[p][Q1, cs],
                           start=True, stop=True, skip_group_check=True)
                        D = {1: wrk.tile([128, 128], F32, tag="D1", name="D1")}
                        nc.vector.scalar_tensor_tensor(
                            D[1][:], pkk[:, 0:128], bmix[:, 0:1], m_d[:],
                            op0=ALU.mult, op1=ALU.mult)

                        # DT via PE transpose; RT0 = I + DT
                        pdt = ps.tile([128, 512], F32, tag="pp")
                        MM(pdt[Q0, 0:64], D[1][Q0, 0:64], ist[Q0, :],
                           start=True, stop=True, skip_group_check=True)
                        MM(pdt[Q1, 64:128], D[1][Q1, 64:128], ist[Q1, :],
                           start=True, stop=True, skip_group_check=True)
                        DT = {1: wrk.tile([128, 128], F32, tag="DT1", name="DT1")}
                        nc.scalar.copy(DT[1][:], pdt[:, 0:128])
                        RT = wrk.tile([128, 128], F32, tag="RT0")
                        nc.vector.tensor_tensor(RT[:], pdt[:, 0:128], i128[:],
                                                op=ALU.add)

                        # power chains + applies
                        for pw in POW:
                            hf_ = pw // 2
                            if pw < 32:
                                pdtp = ps.tile([128, 512], F32, tag="pp")
                                MM(pdtp[Q0, 0:64], D[hf_][Q0, 0:64],
                                   DT[hf_][Q0, 0:64], start=True, stop=True,
                                   skip_group_check=True)
                                MM(pdtp[Q1, 64:128], D[hf_][Q1, 64:128],
                                   DT[hf_][Q1, 64:128], start=True, stop=True,
                                   skip_group_check=True)
                                DT[pw] = wrk.tile([128, 128], F32, tag=f"DT{pw}", name=f"DTp{pw}")
                                nc.scalar.copy(DT[pw][:], pdtp[:, 0:128])
                            pdp = ps.tile([128, 512], F32, tag="pp")
                            MM(pdp[Q0, 0:64], DT[hf_][Q0, 0:64], D[hf_][Q0, 0:64],
                               start=True, stop=True, skip_group_check=True)
                            MM(pdp[Q1, 64:128], DT[hf_][Q1, 64:128],
                               D[hf_][Q1, 64:128], start=True, stop=True,
                               skip_group_check=True)
                            D[pw] = wrk.tile([128, 128], F32, tag=f"D{pw}", name=f"Dp{pw}")
                            nc.vector.tensor_copy(D[pw][:], pdp[:, 0:128])
                            prt = ps.tile([128, 512], F32, tag="pp")
                            MM(prt[Q0, 0:64], D[pw][Q0, 0:64], RT[Q0, 0:64],
                               start=True, stop=True, skip_group_check=True)
                            MM(prt[Q1, 64:128], D[pw][Q1, 64:128], RT[Q1, 64:128],
                               start=True, stop=True, skip_group_check=True)
                            RT2 = wrk.tile([128, 128], F32, tag=f"RT{pw}")
                            nc.vector.tensor_tensor(RT2[:], prt[:, 0:128], RT[:],
                                                    op=ALU.add)
                            RT = RT2

                        # uT (= -u^T) and wT via lhsT = token-major kb/vb
                        puw1 = ps.tile([128, 512], F32, tag="pp")
                        MM(puw1[Q0, 0:64], kb[Q0, 0:64], RT[Q0, 0:64],
                           start=True, stop=True, skip_group_check=True)
                        MM(puw1[Q1, 64:128], kb[Q1, 0:64], RT[Q1, 64:128],
                           start=True, stop=True, skip_group_check=True)
                        uT = wrk.tile([128, 128], F32, tag="uT", bufs=3)
                        nc.scalar.copy(uT[:], puw1[:, 0:128])
                        puw2 = ps.tile([128, 512], F32, tag="pp")
                        MM(puw2[Q0, 0:64], kb[Q0, 64:128], RT[Q0, 0:64],
                           start=True, stop=True, skip_group_check=True)
                        MM(puw2[Q1, 64:128], kb[Q1, 64:128], RT[Q1, 64:128],
                           start=True, stop=True, skip_group_check=True)
                        wT = wrk.tile([128, 128], F32, tag="wT", bufs=3)
                        nc.vector.tensor_copy(wT[:], puw2[:, 0:128])

                        # scan: v_new = w - u@S
                        pvn = ps.tile([128, 512], F32, tag="pp")
                        MM(pvn[Q0, 0:64], uT[0:64, 0:64], S[p][Q0, :],
                           start=True, stop=False, skip_group_check=True)
                        MM(pvn[Q1, 0:64], uT[64:128, 64:128], S[p][Q1, :],
                           start=True, stop=False, skip_group_check=True)
                        MM(pvn[Q0, 0:64], wT[0:64, 0:64], ist[Q0, :],
                           start=False, stop=True, skip_group_check=True)
                        MM(pvn[Q1, 0:64], wT[64:128, 64:128], ist[Q1, :],
                           start=False, stop=True, skip_group_check=True)
                        vn = wrk.tile([128, 64], F32, tag="vn", bufs=3)
                        nc.vector.tensor_copy(vn[:], pvn[:, 0:64])

                        # attnT
                        pat = ps.tile([128, 512], F32, tag="pp")
                        MM(pat[Q0, 0:64], kT[p][Q0, cs], qT[p][Q0, cs],
                           start=True, stop=True, skip_group_check=True)
                        MM(pat[Q1, 64:128], kT[p][Q1, cs], qT[p][Q1, cs],
                           start=True, stop=True, skip_group_check=True)
                        at = wrk.tile([128, 128], F32, tag="at", bufs=3)
                        nc.vector.tensor_tensor(at[:], pat[:, 0:128], m_at[:],
                                                op=ALU.mult)

                        # o = q_sc@S + attn@v_new (q pre-scaled by dk^-0.5)
                        po1 = ps.tile([128, 512], F32, tag="pp")
                        MM(po1[Q0, 0:64], qT[p][Q0, cs], S[p][Q0, :],
                           start=True, stop=False, skip_group_check=True)
                        MM(po1[Q1, 0:64], qT[p][Q1, cs], S[p][Q1, :],
                           start=True, stop=False, skip_group_check=True)
                        MM(po1[:, 0:64], at[:], vn[:], start=False, stop=True,
                           skip_group_check=True)
                        ot = wrk.tile([128, 64], F32, tag="ot", bufs=3)
                        nc.vector.tensor_copy(ot[:], po1[:, 0:64])
                        nc.sync.dma_start(d_o[h1, cs, :], ot[Q0, :])
                        nc.sync.dma_start(d_o[h2, cs, :], ot[Q1, :])

                        # S += k^T v_new
                        pds = ps.tile([128, 512], F32, tag="pp")
                        MM(pds[Q0, 0:64], kb[Q0, 128:192], vn[Q0, :],
                           start=True, stop=True, skip_group_check=True)
                        MM(pds[Q1, 0:64], kb[Q1, 128:192], vn[Q1, :],
                           start=True, stop=True, skip_group_check=True)
                        nc.vector.tensor_tensor(S[p][:], S[p][:], pds[:, 0:64],
                                                op=ALU.add)

    nc.compile()
    return nc


def kernel(hidden_states, Wq, Wk, Wv, Wb):
    hidden_states = np.asarray(hidden_states, np.float32)
    Wq, Wk, Wv, Wb = (np.asarray(x, np.float32) for x in (Wq, Wk, Wv, Wb))
    m_d, m_at, i128, ist, i4 = _masks()

    in_maps = []
    for core in range(N_CORES):
        bi, h0 = core // 4, (core % 4) * 4
        cols = slice(h0 * DK, (h0 + HPC) * DK)
        in_maps.append({
            "hsT": np.ascontiguousarray(hidden_states[bi].T),
            "wq": np.ascontiguousarray(Wq[:, cols]),
            "wkv": np.ascontiguousarray(
                np.concatenate([Wk[:, cols], Wv[:, cols]], axis=1)),
            "wbn": np.ascontiguousarray(-Wb[:, h0:h0 + HPC]),
            "m_d": m_d, "m_at": m_at, "i128": i128, "ist": ist, "i4": i4,
        })

    nc = _build()
    res = bass_utils.run_bass_kernel_spmd(
        nc, in_maps, core_ids=list(range(N_CORES)),
        trace=bool(os.environ.get("BASS_TRACE")))
    LAST_EXEC_NS[0] = res.exec_time_ns

    out = np.empty((B, H, L, DV), np.float32)
    for core in range(N_CORES):
        bi, h0 = core // 4, (core % 4) * 4
        out[bi, h0:h0 + HPC] = res.results[core]["o"]
    return out
